# revision 50
# baseline (speedup 1.0000x reference)
"""GAT (2-layer, PyG-style) Trainium2 Bass kernel, 8-core SPMD.

Sharding: destinations are range-sharded across 8 cores (6250 nodes each).
Each core:
  - computes the full node table h = x @ [W1 | W1@Asrc | W1@Adst] (replicated),
    writes gather tables to HBM (batched DMAs, 8 tiles per transfer; table
    split at the int16 index boundary so A-gathers start early),
  - keeps per-dst-tile attention scores (ad) resident in SBUF,
  - gathers per-edge src rows with SWDGE dma_gather (8-chunk windows, one
    SWDGE queue; the ucode descriptor ring is hard-capped at 1024),
  - computes per-edge dst scores via TensorE matmuls against host-built
    TRANSPOSED 0/1 scatter blocks (S01T) -- no dst gather,
  - computes edge scores  e = leakyrelu(as[src]+ad[dst]),  w = exp(e)
    (max-subtraction dropped: it cancels exactly in the softmax ratio),
  - aggregates  out[d] = (sum_e S01[e,d] * w_e * h[src_e]) / (sum_e S01[e,d] * w_e)
    via PSUM-accumulated TensorE matmuls against host-built 0/1 scatter blocks,
  - applies bias+ELU, computes layer-2 node rows, AllGathers them (1.6MB/rank),
  - layer-2 dst scores ad2[dst_e] are computed during the layer-1 pass while
    each group's S01T blocks are still SBUF-resident (stashed in pss2_all), so
    the layer-2 pass skips all S01T loads (-15.4MB HBM traffic per core),
  - repeats the gather/aggregate phase for layer 2 and writes its output shard.

All indices / scatter blocks / paddings are host-precomputed per core and fed
as per-core input tensors, so one SPMD NEFF serves all 8 cores.
"""

import math
from dataclasses import dataclass, field

import numpy as np
import ml_dtypes

BF16 = ml_dtypes.bfloat16
FP8 = ml_dtypes.float8_e4m3

P = 128  # partitions / tile edge

# HW bisection: "A"=node tables only, "C"=+layer1 agg+allgather, "D"=full
BUILD_STAGE = "D"


@dataclass
class Cfg:
    n_nodes: int = 50000
    n_edges: int = 800000  # before self loops
    f_in: int = 128
    heads: int = 8
    hid: int = 32
    n_cores: int = 8
    group_tiles: int = 4  # dst tiles per gather group
    split: int = 32768  # int16 index split point
    neg_slope: float = 0.2
    window: int = 8  # gather chunks per dma_gather call
    nqueues: int = 1  # SWDGE queues
    single_packet: bool = True  # pack each gather call into one SDMA packet
    scratch: int = 16384  # dynamic DMA scratch bytes (ring = scratch/16 descs)
    nb: int = 4  # node-table tiles per batched DMA

    @property
    def shard(self):
        return self.n_nodes // self.n_cores

    @property
    def hc(self):
        return self.heads * self.hid  # 256

    @property
    def n_tiles(self):
        return math.ceil(self.shard / P)  # dst tiles per core

    @property
    def nt1(self):
        return math.ceil(self.n_nodes / P)  # node-table tiles

    @property
    def n_pad(self):
        return self.nt1 * P

    @property
    def shard_pad(self):
        return self.n_tiles * P


# ------------------------------------------------------------ host preprocess


@dataclass
class Plan:
    """Structure shared by all cores (uniform) + per-core tensor data."""

    CA: list = field(default_factory=list)  # A-chunks per tile (maxed over cores)
    CB: list = field(default_factory=list)
    groups: list = field(default_factory=list)  # per group: list of tile ids
    g_nA: list = field(default_factory=list)
    g_nB: list = field(default_factory=list)
    g_chunk0: list = field(default_factory=list)
    g_slot0: list = field(default_factory=list)  # bufh slot offsets (incl self)
    k_tot: int = 0  # A/B chunks (S01/IDXA entries)
    slot_tot: int = 0  # bufh slots incl per-tile self chunks
    data: list = field(default_factory=list)  # per-core input arrays
    ids: np.ndarray | None = None  # [n_cores, shard] node id per output row


def _wrap16(idx: np.ndarray) -> np.ndarray:
    """[n] -> [128, n/16] int16 gather-index layout (16-wrapped, x8 replicated)."""
    n = idx.shape[0]
    assert n % 16 == 0
    a = idx.astype(np.int16).reshape(n // 16, 16).T  # [16, n/16]
    return np.tile(a, (8, 1)).copy()


def _balanced_assign(dA: np.ndarray, dB: np.ndarray, cfg: Cfg):
    """Relabel nodes into balanced dst buckets (8 cores x 49 tiles).

    Nodes are snake-dealt (sorted desc by (dB, dA) in-degree) so per-bucket
    A/B edge sums are near-uniform, removing both the max-over-cores chunk
    padding and most per-tile rounding waste (k_tot 940 -> 880).

    The relabeled row is ALSO the gather-table row for both layers (the
    layer-2 table comes out of the AllGather in assignment order). To keep
    the int16 A/B table split valid for both layers, nodes with id < split
    must stay in rows < split: pool-A nodes are dealt into the low-row slots
    of each bucket, pool-B into the rest.
    """
    N = cfg.n_nodes
    nt, ncr = cfg.n_tiles, cfg.n_cores
    tail = cfg.shard - (nt - 1) * P
    nbuck = ncr * nt
    cap = np.full(nbuck, P, np.int64)
    cap[nt - 1 :: nt] = tail  # last tile of each core holds the remainder
    # global row of each bucket's position 0, and its pool-A slot quota
    r0 = np.array(
        [(b // nt) * cfg.shard + (b % nt) * P for b in range(nbuck)], np.int64
    )
    capA = np.clip(cfg.split - r0, 0, cap)
    capB = cap - capA

    node_core = np.empty(N, np.int32)
    node_tile = np.empty(N, np.int32)
    node_pos = np.empty(N, np.int32)
    order = np.lexsort((dA, dB))[::-1]

    def deal(nodes, quota, pos0):
        fill = np.zeros(nbuck, np.int64)
        active = [b for b in range(nbuck) if quota[b] > 0]
        i = 0
        fwd = True
        while i < len(nodes):
            seq = active if fwd else active[::-1]
            for b in seq:
                if i >= len(nodes):
                    break
                n = nodes[i]
                node_core[n] = b // nt
                node_tile[n] = b % nt
                node_pos[n] = pos0[b] + fill[b]
                fill[b] += 1
                i += 1
            active = [b for b in active if fill[b] < quota[b]]
            fwd = not fwd
        assert (fill == quota).all()

    deal(order[order < cfg.split], capA, np.zeros(nbuck, np.int64))
    deal(order[order >= cfg.split], capB, capA)
    return node_core, node_tile, node_pos


def preprocess(edge_index: np.ndarray, cfg: Cfg) -> Plan:
    N = cfg.n_nodes
    # self-loops are NOT part of the gathered edge streams: each tile's 128
    # self edges are consecutive table rows, loaded by one contiguous DMA
    # into dedicated "self" chunk slots (no per-row SWDGE descriptors)
    src = edge_index[0].astype(np.int64)
    dst = edge_index[1].astype(np.int64)

    plan = Plan()
    ncores = cfg.n_cores
    shard = cfg.shard
    cdiv = lambda a, b: -(-a // b)

    dA = np.bincount(dst[src < cfg.split], minlength=N)
    dB = np.bincount(dst[src >= cfg.split], minlength=N)
    node_core, node_tile, node_pos = _balanced_assign(dA, dB, cfg)
    # per-core output row order: node at (c, t, p) -> OUT row t*P + p
    ids = np.empty((ncores, shard), np.int64)
    rows = node_tile.astype(np.int64) * P + node_pos
    for c in range(ncores):
        m = node_core == c
        ids[c, rows[m]] = np.nonzero(m)[0]
    plan.ids = ids
    # gather tables (both layers) are stored in assignment-row order, so
    # gather indices are the relabeled rows; pool-preserving assignment keeps
    # src<split <=> row<split, so one A/B chunk split serves both layers
    row2 = node_core.astype(np.int64) * shard + rows
    src = row2[src]

    e_core = node_core[dst]
    e_tile = node_tile[dst]
    e_pos = node_pos[dst]
    per_core = []
    for c in range(ncores):
        m = e_core == c
        s_c = src[m]
        t_c = e_tile[m]
        p_c = e_pos[m]
        order = np.argsort(t_c * P + p_c, kind="stable")
        s_c, t_c, p_c = s_c[order], t_c[order], p_c[order]
        tiles = []
        for t in range(cfg.n_tiles):
            tm = t_c == t
            s_t, d_t = s_c[tm], p_c[tm]
            a = s_t < cfg.split
            tiles.append((s_t[a], d_t[a], s_t[~a], d_t[~a]))
        per_core.append(tiles)

    for t in range(cfg.n_tiles):
        plan.CA.append(max(cdiv(len(per_core[c][t][0]), P) for c in range(ncores)))
        plan.CB.append(max(cdiv(len(per_core[c][t][2]), P) for c in range(ncores)))

    for g0 in range(0, cfg.n_tiles, cfg.group_tiles):
        plan.groups.append(list(range(g0, min(g0 + cfg.group_tiles, cfg.n_tiles))))
    k = 0
    s = 0
    for g in plan.groups:
        plan.g_chunk0.append(k)
        plan.g_slot0.append(s)
        plan.g_nA.append(sum(plan.CA[t] for t in g))
        plan.g_nB.append(sum(plan.CB[t] for t in g))
        k += plan.g_nA[-1] + plan.g_nB[-1]
        s += plan.g_nA[-1] + plan.g_nB[-1] + len(g)  # + one self chunk per tile
    plan.k_tot = k
    plan.slot_tot = s

    for c in range(ncores):
        idxA = []
        s01 = np.zeros((plan.k_tot, P, P), dtype=FP8)
        for gi, g in enumerate(plan.groups):
            k0 = plan.g_chunk0[gi]
            nA = plan.g_nA[gi]
            a_off = 0
            b_off = 0
            gA_src, gB_src = [], []
            for t in g:
                sA, dA, sB, dB = per_core[c][t]
                la, lb = plan.CA[t] * P, plan.CB[t] * P
                sA_p = np.concatenate([sA, np.zeros(la - len(sA), np.int64)])
                sB_p = np.concatenate(
                    [sB - cfg.split, np.zeros(lb - len(sB), np.int64)]
                )
                gA_src.append(sA_p)
                gB_src.append(sB_p)
                if len(sA):
                    jj = np.arange(len(sA))
                    s01[k0 + a_off + jj // P, jj % P, dA] = 1.0
                if len(sB):
                    jj = np.arange(len(sB))
                    s01[k0 + nA + b_off + jj // P, jj % P, dB] = 1.0
                a_off += plan.CA[t]
                b_off += plan.CB[t]
            idxA.append(np.concatenate(gA_src + gB_src))
        # S01T (edge->dst score scatter) is built WITHOUT the fake entries:
        # it must map each edge slot to exactly its own dst column, otherwise
        # real edges sharing a slot with a fake entry get corrupted scores.
        s01T = np.ascontiguousarray(s01.transpose(0, 2, 1))  # [K, d, e]
        # fake entries so padded dst columns get denom > 0 (aggregation only)
        for gi, g in enumerate(plan.groups):
            k0 = plan.g_chunk0[gi]
            nA = plan.g_nA[gi]
            a_off = 0
            b_off = 0
            for t in g:
                width = min(cfg.shard - t * P, P)
                if width < P and plan.CA[t] + plan.CB[t] > 0:
                    kf = (k0 + a_off) if plan.CA[t] > 0 else (k0 + nA + b_off)
                    for d_pad in range(width, P):
                        s01[kf, (d_pad - width) % P, d_pad] = 1.0
                a_off += plan.CA[t]
                b_off += plan.CB[t]
        cat = lambda xs: (
            np.concatenate([_wrap16(x) for x in xs if len(x)], axis=1)
            if any(len(x) for x in xs)
            else np.zeros((128, 0), np.int16)
        )
        plan.data.append(
            {
                "IDXA": cat(idxA),
                "S01": np.ascontiguousarray(s01.transpose(1, 0, 2)),  # [128e,K,128d]
                "S01T": np.ascontiguousarray(s01T.transpose(1, 0, 2)),  # [128d,K,128e]
            }
        )
    return plan


def prep_weights(inputs: dict, cfg: Cfg):
    W1 = np.asarray(inputs["W1"], np.float32)
    a_s1 = np.asarray(inputs["att_src1"], np.float32)
    a_d1 = np.asarray(inputs["att_dst1"], np.float32)
    W2 = np.asarray(inputs["W2"], np.float32)
    a_s2 = np.asarray(inputs["att_src2"], np.float32)
    a_d2 = np.asarray(inputs["att_dst2"], np.float32)
    H, C = cfg.heads, cfg.hid
    W1r = W1.reshape(cfg.f_in, H, C)
    w1as = np.einsum("fhc,hc->fh", W1r, a_s1)
    w1ad = np.einsum("fhc,hc->fh", W1r, a_d1)
    W1p = np.concatenate([W1, w1as, w1ad], axis=1).astype(BF16)  # [F, HC+16]
    w2as = W2 @ a_s2[0]
    w2ad = W2 @ a_d2[0]
    W2p = np.concatenate([W2, w2as[:, None], w2ad[:, None]], axis=1).astype(BF16)
    b1rep = np.tile(np.asarray(inputs["b1"], np.float32)[None, :], (P, 1))
    b2rep = np.tile(np.asarray(inputs["b2"], np.float32)[None, :], (P, 1))
    return W1p, W2p, b1rep.astype(np.float32), b2rep.astype(np.float32)


# ---------------------------------------------------------------- bass kernel


def build_kernel(
    cfg: Cfg,
    plan: Plan,
    sim_1core: bool = False,
    phase_marks: dict | None = None,
    repeat: int = 1,  # unroll the whole body N times (bench-only: measures
    # device-serial time per body without per-dispatch RPC overhead)
):
    from contextlib import ExitStack

    import concourse.bacc as bacc
    import concourse.bass as bass
    import concourse.mybir as mybir
    import concourse.tile as tile

    fp32 = mybir.dt.float32
    bf16 = mybir.dt.bfloat16
    fp8 = mybir.dt.float8e4
    i16 = mybir.dt.int16
    AF = mybir.ActivationFunctionType
    OP = mybir.AluOpType

    HC = cfg.hc  # 256
    HCX = HC + 16
    H = cfg.heads
    HID = cfg.hid
    NTAB = cfg.n_pad
    SH = cfg.shard
    SHP = cfg.shard_pad
    T2W = 128  # layer-2 table row width (256B rows): [h2 | as2 | ad2 | pad]
    NKW2 = HC // P  # 2 chunks for the layer-2 prep matmul
    NT = cfg.n_tiles

    nc = bacc.Bacc(
        "TRN2",
        num_devices=1 if sim_1core else cfg.n_cores,
        num_swdge_queues=cfg.nqueues,
        dynamic_dma_scratch_size=cfg.scratch,
        name="gat8",
    )

    def mark(name):
        if phase_marks is not None:
            phase_marks[name] = len(nc.inst_map)

    xT = nc.dram_tensor("xT", [P, NTAB], bf16, kind="ExternalInput")
    xTown = nc.dram_tensor("xTown", [P, SHP], bf16, kind="ExternalInput")
    W1p = nc.dram_tensor("W1p", [cfg.f_in, HCX], bf16, kind="ExternalInput")
    W2p = nc.dram_tensor("W2p", [HC, HID + 2], bf16, kind="ExternalInput")
    b1rep = nc.dram_tensor("b1rep", [P, HC], fp32, kind="ExternalInput")
    b2rep = nc.dram_tensor("b2rep", [P, HID], fp32, kind="ExternalInput")
    identity = nc.dram_tensor("identity", [P, P], bf16, kind="ExternalInput")
    d0 = plan.data[0]
    IDXA = nc.dram_tensor("IDXA", list(d0["IDXA"].shape), i16, kind="ExternalInput")
    S01 = nc.dram_tensor("S01", [P, plan.k_tot, P], fp8, kind="ExternalInput")
    S01T = nc.dram_tensor("S01T", [P, plan.k_tot, P], fp8, kind="ExternalInput")
    OUT = nc.dram_tensor("out", [SH, HID], fp32, kind="ExternalOutput")

    with tile.TileContext(nc) as tc, ExitStack() as ctx:
        _regs = {}

        def nreg(v):
            if v not in _regs:
                _regs[v] = nc.gpsimd.to_reg(v)
            return _regs[v]

        sb = ctx.enter_context(tc.tile_pool(name="sb", bufs=2))
        sb1 = ctx.enter_context(tc.tile_pool(name="sb1", bufs=1))
        psA = ctx.enter_context(tc.tile_pool(name="psA", bufs=2, space="PSUM"))
        psB = ctx.enter_context(tc.tile_pool(name="psB", bufs=2, space="PSUM"))
        dram = ctx.enter_context(tc.tile_pool(name="dram", bufs=1, space="DRAM"))

        # node table split at the int16 index boundary so A-gathers can start
        # before the B half is written
        T1xA = dram.tile([cfg.split, 384], bf16, tag="T1xA")  # [h|as|ad|pad]
        T1xB = dram.tile([NTAB - cfg.split, 384], bf16, tag="T1xB")
        T1own = dram.tile([SHP, 384], bf16, tag="T1own")  # own-shard rows for
        # the contiguous self-chunk loads (static per-core address)
        T2sh = dram.tile([SH, T2W], bf16, tag="T2sh")
        T2full = dram.tile([cfg.n_nodes, T2W], bf16, tag="T2full")

        # constants
        w1_sb = sb1.tile([cfg.f_in, HCX], bf16, tag="w1")
        nc.sync.dma_start(w1_sb[:], W1p[:])
        w2_sb = sb1.tile([P, NKW2 * (HID + 2)], bf16, tag="w2")
        nc.sync.dma_start(
            w2_sb[:].rearrange("p (a n) -> p a n", a=NKW2),
            W2p[:].rearrange("(a p) n -> p a n", p=P),
        )
        w2_3 = w2_sb[:].rearrange("p (a n) -> p a n", a=NKW2)
        b1_sb = sb1.tile([P, HC], fp32, tag="b1")
        nc.sync.dma_start(b1_sb[:], b1rep[:])
        b2_sb = sb1.tile([P, HID], fp32, tag="b2")
        nc.sync.dma_start(b2_sb[:], b2rep[:])
        id_sb = sb1.tile([P, P], bf16, tag="id")
        nc.sync.dma_start(id_sb[:], identity[:])
        # fp8 identity for the self-chunk agg matmuls: they accumulate in the
        # same PSUM group as the fp8 scatter-block matmuls, and the lhsT dtype
        # must stay uniform within one accumulation group
        id8 = sb1.tile([P, P], fp8, tag="id8")
        nc.scalar.copy(id8[:], id_sb[:])

        # per-dst-tile attention scores, SBUF-resident
        adt1 = sb1.tile([P, NT * H], bf16, tag="adt1")  # ad per own node, layer 1
        adt2 = sb1.tile([P, NT], bf16, tag="adt2")  # ad2 per own node, layer 2
        # per-edge-chunk layer-2 dst scores ad2[dst_e], computed during the
        # layer-1 pass while the group's S01T blocks are still SBUF-resident
        # (saves reloading 15.4MB of S01T in the layer-2 pass)
        pss2_all = sb1.tile([P, plan.slot_tot], bf16, tag="pss2")

        # ---------------- phase 1: node table (batched DMAs) ----------------
        def phase1():
            NB = cfg.nb
            for i0 in range(0, cfg.nt1, NB):
                nb = min(NB, cfg.nt1 - i0)
                xt = sb.tile([P, NB * P], bf16, tag="xt")
                nc.sync.dma_start(xt[:, : nb * P], xT[:, i0 * P : (i0 + nb) * P])
                stg = sb.tile([P, NB * HCX], bf16, tag="stg1")
                stg3 = stg[:].rearrange("p (a e) -> p a e", a=NB)
                for j in range(nb):
                    pt = psA.tile([P, HCX], fp32, tag="p1")
                    nc.tensor.matmul(
                        out=pt[:],
                        lhsT=xt[:, j * P : (j + 1) * P],
                        rhs=w1_sb[:],
                        start=True,
                        stop=True,
                    )
                    if j % 2 == 0:
                        nc.vector.tensor_copy(stg3[:, j, :], pt[:])
                    else:
                        nc.scalar.copy(stg3[:, j, :], pt[:])
                r0 = i0 * P
                if r0 < cfg.split:
                    dst_tab = T1xA[r0 : r0 + nb * P, :HCX]
                else:
                    dst_tab = T1xB[r0 - cfg.split : r0 - cfg.split + nb * P, :HCX]
                # table writes on the Act HWDGE queue: the xt loads keep the
                # SP queue busy, so splitting queues overlaps load and store
                nc.scalar.dma_start(
                    dst_tab.rearrange("(a p) e -> p a e", p=P),
                    stg3[:, :nb, :],
                )

            # phase 1b: own-shard node rows -> T1own (for the contiguous
            # self-chunk loads) + dst scores (ad) kept in SBUF
            for i0 in range(0, NT, NB):
                nb = min(NB, NT - i0)
                xo = sb.tile([P, NB * P], bf16, tag="xo")
                nc.sync.dma_start(xo[:, : nb * P], xTown[:, i0 * P : (i0 + nb) * P])
                stg = sb.tile([P, NB * HCX], bf16, tag="stg1")
                stg3 = stg[:].rearrange("p (a e) -> p a e", a=NB)
                for j in range(nb):
                    pt = psA.tile([P, HCX], fp32, tag="p1")
                    nc.tensor.matmul(
                        out=pt[:],
                        lhsT=xo[:, j * P : (j + 1) * P],
                        rhs=w1_sb[:],
                        start=True,
                        stop=True,
                    )
                    if j % 2 == 0:
                        nc.vector.tensor_copy(stg3[:, j, :], pt[:])
                    else:
                        nc.scalar.copy(stg3[:, j, :], pt[:])
                    nc.scalar.copy(
                        adt1[:, (i0 + j) * H : (i0 + j + 1) * H],
                        pt[:, HC + H : HCX],
                    )
                nc.scalar.dma_start(
                    T1own[i0 * P : (i0 + nb) * P, :HCX].rearrange(
                        "(a p) e -> p a e", p=P
                    ),
                    stg3[:, :nb, :],
                )

        # ------------- layer-1 per-tile epilogue: bias, ELU, layer-2 rows ----
        def epilogue1_group(g, o1slab, r2slab3):
            """Group-level bias+ELU, then per-tile layer-2 row prep."""
            nt_g = len(g)
            w_cols = nt_g * HC
            y = sb.tile([P, nt_g * HC], fp32, tag="ep_y")
            nc.vector.tensor_tensor(
                out=y[:, :w_cols].rearrange("p (a e) -> p a e", a=nt_g),
                in0=o1slab[:, :w_cols].rearrange("p (a e) -> p a e", a=nt_g),
                in1=b1_sb[:].rearrange("p (u e) -> p u e", u=1).to_broadcast(
                    [P, nt_g, HC]
                ),
                op=OP.add,
            )
            mn = sb.tile([P, nt_g * HC], fp32, tag="ep_mn")
            nc.vector.tensor_scalar_min(mn[:, :w_cols], y[:, :w_cols], 0.0)
            nc.scalar.activation(mn[:, :w_cols], mn[:, :w_cols], AF.Exp)
            nc.vector.tensor_scalar_max(y[:, :w_cols], y[:, :w_cols], 0.0)
            nc.vector.tensor_tensor(
                out=y[:, :w_cols], in0=y[:, :w_cols], in1=mn[:, :w_cols], op=OP.add
            )
            elu_bf = sb.tile([P, nt_g * HC], bf16, tag="ep_bf")
            nc.vector.tensor_scalar_add(elu_bf[:, :w_cols], y[:, :w_cols], -1.0)
            for jj, t in enumerate(g):
                eluT = sb.tile([P, HC], bf16, tag="ep_eT")
                for j in range(NKW2):
                    ptT = psB.tile([P, P], bf16, tag="p2")
                    nc.tensor.transpose(
                        out=ptT[:],
                        in_=elu_bf[:, jj * HC + j * P : jj * HC + (j + 1) * P],
                        identity=id_sb[:],
                    )
                    nc.scalar.copy(eluT[:, j * P : (j + 1) * P], ptT[:])
                p2 = psB.tile([P, HID + 2], fp32, tag="p2")
                for j in range(NKW2):
                    nc.tensor.matmul(
                        out=p2[:],
                        lhsT=eluT[:, j * P : (j + 1) * P],
                        rhs=w2_3[:, j, :],
                        start=(j == 0),
                        stop=(j == NKW2 - 1),
                    )
                nc.vector.tensor_copy(r2slab3[:, jj, :], p2[:])
                nc.scalar.copy(adt2[:, t : t + 1], p2[:, HID + 1 : HID + 2])

        # ---------------- shared gather/aggregate phase ----------------
        qctr = [0]

        def agg_layer(layer):
            elem_h = 384 if layer == 1 else T2W
            nhead = H if layer == 1 else 1
            rhs_w = HC if layer == 1 else HID  # message width

            for gi, g in enumerate(plan.groups):
                nA, nBg = plan.g_nA[gi], plan.g_nB[gi]
                ng = nA + nBg  # A/B chunks (have S01 blocks + gather idxs)
                nt_g = len(g)
                ngs = ng + nt_g  # + one self chunk per tile (slots at the end)
                k0 = plan.g_chunk0[gi]
                s0g = plan.g_slot0[gi]

                bufh = sb.tile([P, ngs * elem_h], bf16, tag="bufh")
                bufh3 = bufh[:].rearrange("p (k e) -> p k e", e=elem_h)

                def win_gather(out3, table_ap, idx_tile, c0, n_chunks, elem):
                    for w0 in range(0, n_chunks, cfg.window):
                        wn = min(cfg.window, n_chunks - w0)
                        nc.gpsimd.dma_gather(
                            out_ap=out3[:, c0 + w0 : c0 + w0 + wn, :],
                            in_ap=table_ap,
                            idxs_ap=idx_tile[:, (c0 + w0) * 8 : (c0 + w0 + wn) * 8],
                            num_idxs=wn * P,
                            num_idxs_reg=nreg(wn * P),
                            elem_size=elem,
                            queue_num=qctr[0] % cfg.nqueues,
                            single_packet=cfg.single_packet,
                        )
                        qctr[0] += 1

                ih = sb.tile([P, ng * 8], i16, tag="ia")
                a0 = sum(plan.g_nA[j] + plan.g_nB[j] for j in range(gi)) * 8
                nc.sync.dma_start(ih[:], IDXA[:, a0 : a0 + ng * 8])
                tabA = T1xA if layer == 1 else T2full
                tabB = T1xB if layer == 1 else T2full[cfg.split :, :]
                if nA:
                    win_gather(bufh3, tabA[:, :] if layer == 1 else tabA[:, :], ih, 0, nA, elem_h)
                if nBg:
                    win_gather(bufh3, tabB[:, :] if layer == 1 else tabB, ih, nA, nBg, elem_h)
                # self chunks: contiguous own-shard rows, plain HWDGE DMA
                t0g = g[0]
                if layer == 1:
                    nc.scalar.dma_start(
                        bufh3[:, ng : ng + nt_g, :],
                        T1own[t0g * P : (t0g + nt_g) * P, :].rearrange(
                            "(a p) e -> p a e", p=P
                        ),
                    )
                else:
                    for jj, t in enumerate(g):
                        width = min(SH - t * P, P)
                        nc.scalar.dma_start(
                            bufh3[:width, ng + jj, :],
                            T2sh[t * P : t * P + width, :],
                        )

                s01_sb = sb.tile([P, ng * P], fp8, tag="s01")
                nc.sync.dma_start(s01_sb[:], S01[:, k0 : k0 + ng, :])
                s01_3 = s01_sb[:].rearrange("p (k d) -> p k d", d=P)
                if layer == 1:
                    s01T_sb = sb.tile([P, ng * P], fp8, tag="s01T")
                    nc.scalar.dma_start(s01T_sb[:], S01T[:, k0 : k0 + ng, :])
                    s01T_3 = s01T_sb[:].rearrange("p (k e) -> p k e", e=P)

                    # dst scores via S01T matmuls: psS[e, k*nh:] = S01T_k^T @ adt[t]
                    # (self chunks: identity scatter -> a PE copy of adt1[t])
                    pss = psA.tile([P, ngs * nhead], fp32, tag="pagg")
                    a_off, b_off = 0, 0
                    for jj, t in enumerate(g):
                        chunks = [a_off + j for j in range(plan.CA[t])] + [
                            nA + b_off + j for j in range(plan.CB[t])
                        ]
                        for k in chunks:
                            nc.tensor.matmul(
                                out=pss[:, k * nhead : (k + 1) * nhead],
                                lhsT=s01T_3[:, k, :],
                                rhs=adt1[:, t * H : (t + 1) * H],
                                start=True,
                                stop=True,
                            )
                        kS = ng + jj
                        nc.tensor.matmul(
                            out=pss[:, kS * nhead : (kS + 1) * nhead],
                            lhsT=id_sb[:],
                            rhs=adt1[:, t * H : (t + 1) * H],
                            start=True,
                            stop=True,
                        )
                        a_off += plan.CA[t]
                        b_off += plan.CB[t]
                    pss_ap = pss[:].rearrange("p (k h) -> p k h", h=nhead)
                else:
                    # layer-2 dst scores were stashed during the layer-1 pass
                    pss_ap = pss2_all[:, s0g : s0g + ngs].rearrange(
                        "p (k h) -> p k h", h=1
                    )

                # scores: s = as[src] + ad[dst]; w = exp(leakyrelu(s))
                nsc = ngs * nhead
                s_f = sb.tile([P, nsc], fp32, tag="s_f")
                if layer == 1:
                    as_ap = bufh3[:, :, HC : HC + H]
                else:
                    as_ap = bufh3[:, :, HID : HID + 1]
                nc.vector.tensor_tensor(
                    out=s_f[:],
                    in0=as_ap,
                    in1=pss_ap,
                    op=OP.add,
                )
                s_lr = sb.tile([P, nsc], fp32, tag="s_lr")
                nc.scalar.mul(s_lr[:], s_f[:], cfg.neg_slope)
                nc.vector.tensor_tensor(out=s_f[:], in0=s_lr[:], in1=s_f[:], op=OP.max)
                # w written pair-duplicated so the premultiply's broadcast
                # operand has an innermost stride-1 pair (DVE 2x mode)
                w2x = sb.tile([P, nsc * 2], bf16, tag="w2x")
                nc.scalar.activation(
                    w2x[:].rearrange("p (k two) -> p k two", two=2),
                    s_f[:].rearrange("p (k u) -> p k u", u=1).to_broadcast(
                        [P, nsc, 2]
                    ),
                    AF.Exp,
                )
                # copy w into the padding columns of the gathered rows so the
                # aggregation matmul also produces softmax denominators
                wcol = 256 if layer == 1 else 33
                nc.scalar.copy(
                    bufh3[:, :, wcol : wcol + nhead],
                    w2x[:].rearrange("p (k two) -> p k two", two=2)[:, :, 0].rearrange(
                        "p (k h) -> p k h", h=nhead
                    ),
                )

                # premultiply gathered message rows by w (in place, 2x mode)
                mw_io = bufh3[:, :, : (HC if layer == 1 else HID)].rearrange(
                    "p k (h x two) -> p k h x two", h=nhead, two=2
                )
                w_b = w2x[:].rearrange(
                    "p (k h u two) -> p k h u two", h=nhead, u=1, two=2
                ).to_broadcast([P, ngs, nhead, HID // 2, 2])
                nc.vector.tensor_tensor(out=mw_io, in0=mw_io, in1=w_b, op=OP.mult)

                a_off, b_off = 0, 0
                rw = wcol + nhead  # matmul rhs width incl junk + w cols
                nt_g = len(g)
                if layer == 1:
                    r2slab = sb.tile([P, nt_g * (HID + 2)], fp32, tag="r2slab")
                    r2slab3 = r2slab[:].rearrange("p (a e) -> p a e", a=nt_g)
                    o1slab = sb.tile([P, nt_g * HC], fp32, tag="o1slab")
                    o1slab3 = o1slab[:].rearrange(
                        "p (a h c) -> p a h c", a=nt_g, c=HID
                    )
                else:
                    oslab = sb.tile([P, nt_g * HID], fp32, tag="oslab")
                    oslab3 = oslab[:].rearrange("p (a e) -> p a e", a=nt_g)
                for jj, t in enumerate(g):
                    pt = psA.tile([P, rw], fp32, tag="pagg")
                    chunks = [a_off + j for j in range(plan.CA[t])] + [
                        nA + b_off + j for j in range(plan.CB[t])
                    ]
                    for ci, k in enumerate(chunks):
                        nc.tensor.matmul(
                            out=pt[:],
                            lhsT=s01_3[:, k, :],
                            rhs=bufh3[:, k, :rw],
                            start=(ci == 0),
                            stop=False,
                        )
                    # self chunk: identity scatter (slot p -> dst col p); for
                    # the layer-2 tail tile only `width` rows of T2sh exist,
                    # so contract over [:width] to exclude stale SBUF rows
                    kS = ng + jj
                    width = min(SH - t * P, P)
                    wlim = width if (layer == 2 and width < P) else P
                    nc.tensor.matmul(
                        out=pt[:],
                        lhsT=id8[:wlim, :],
                        rhs=bufh3[:wlim, kS, :rw],
                        start=(len(chunks) == 0),
                        stop=True,
                    )
                    a_off += plan.CA[t]
                    b_off += plan.CB[t]

                    den_r = sb.tile([P, nhead], fp32, tag="denr")
                    nc.vector.reciprocal(den_r[:], pt[:, wcol : wcol + nhead])
                    if layer == 1:
                        nc.vector.tensor_tensor(
                            out=o1slab3[:, jj, :, :],
                            in0=pt[:, :rhs_w].rearrange("p (h c) -> p h c", c=HID),
                            in1=den_r[:].to_broadcast([P, nhead, HID]),
                            op=OP.mult,
                        )
                    else:
                        nc.vector.tensor_tensor(
                            out=oslab3[:, jj, :],
                            in0=pt[:, :HID],
                            in1=den_r[:].to_broadcast([P, HID]),
                            op=OP.mult,
                        )
                        nc.vector.tensor_tensor(
                            out=oslab3[:, jj, :],
                            in0=oslab3[:, jj, :],
                            in1=b2_sb[:],
                            op=OP.add,
                        )
                # batched per-group stores
                t0g = g[0]
                rows = min(SH - t0g * P, nt_g * P)
                if layer == 1:
                    epilogue1_group(g, o1slab, r2slab3)
                    # stash layer-2 dst scores for this group's chunks while
                    # its S01T blocks are still resident
                    p2s = psB.tile([P, ng], fp32, tag="p2")
                    a_off2, b_off2 = 0, 0
                    for jj, t in enumerate(g):
                        chunks2 = [a_off2 + j for j in range(plan.CA[t])] + [
                            nA + b_off2 + j for j in range(plan.CB[t])
                        ]
                        for k in chunks2:
                            nc.tensor.matmul(
                                out=p2s[:, k : k + 1],
                                lhsT=s01T_3[:, k, :],
                                rhs=adt2[:, t : t + 1],
                                start=True,
                                stop=True,
                            )
                        a_off2 += plan.CA[t]
                        b_off2 += plan.CB[t]
                        # self chunk: ad2[dst=self] is just adt2[t] itself
                        nc.scalar.copy(
                            pss2_all[:, s0g + ng + jj : s0g + ng + jj + 1],
                            adt2[:, t : t + 1],
                        )
                    nc.scalar.copy(pss2_all[:, s0g : s0g + ng], p2s[:])
                    r2bf = sb.tile([P, nt_g * (HID + 2)], bf16, tag="r2bf")
                    nc.vector.tensor_copy(r2bf[:], r2slab[:])
                    r2bf3 = r2bf[:].rearrange("p (a e) -> p a e", a=nt_g)
                    if rows % P == 0:
                        nc.sync.dma_start(
                            T2sh[t0g * P : t0g * P + rows, : HID + 2].rearrange(
                                "(a p) e -> p a e", p=P
                            ),
                            r2bf3[:, : rows // P, :],
                        )
                    else:
                        nfull = rows // P
                        if nfull:
                            nc.sync.dma_start(
                                T2sh[t0g * P : t0g * P + nfull * P, : HID + 2].rearrange(
                                    "(a p) e -> p a e", p=P
                                ),
                                r2bf3[:, :nfull, :],
                            )
                        rem = rows - nfull * P
                        nc.sync.dma_start(
                            T2sh[
                                t0g * P + nfull * P : t0g * P + rows, : HID + 2
                            ],
                            r2bf3[:rem, nfull, :],
                        )
                else:
                    if rows % P == 0:
                        nc.sync.dma_start(
                            OUT[t0g * P : t0g * P + rows, :].rearrange(
                                "(a p) e -> p a e", p=P
                            ),
                            oslab3[:, : rows // P, :],
                        )
                    else:
                        nfull = rows // P
                        if nfull:
                            nc.sync.dma_start(
                                OUT[t0g * P : t0g * P + nfull * P, :].rearrange(
                                    "(a p) e -> p a e", p=P
                                ),
                                oslab3[:, :nfull, :],
                            )
                        rem = rows - nfull * P
                        nc.sync.dma_start(
                            OUT[t0g * P + nfull * P : t0g * P + rows, :],
                            oslab3[:rem, nfull, :],
                        )

        for _rep in range(repeat):
            phase1()
            mark("node_tables")
            if BUILD_STAGE != "A":
                agg_layer(1)
            mark("agg1")

            if BUILD_STAGE in ("C", "D"):
                if cfg.n_cores > 1 and not sim_1core:
                    nc.gpsimd.collective_compute(
                        "AllGather",
                        OP.bypass,
                        replica_groups=[list(range(cfg.n_cores))],
                        ins=[T2sh.opt()],
                        outs=[T2full.opt()],
                    )
                else:
                    nc.sync.dma_start(T2full[:SH, :], T2sh[:, :])
            mark("allgather")

            if BUILD_STAGE == "D":
                agg_layer(2)
                mark("agg2")
            else:
                stg0 = sb.tile([P, HID], fp32, tag="dumm")
                for t in range(NT):
                    rows = min(SH - t * P, P)
                    nc.vector.tensor_copy(stg0[:rows, :], b2_sb[:rows, :])
                    nc.sync.dma_start(OUT[t * P : t * P + rows, :], stg0[:rows, :])

    nc.compile()
    return nc


# -------------------------------------------------------------------- driver


def make_in_maps(inputs: dict, cfg: Cfg, plan: Plan):
    x = np.asarray(inputs["x"], np.float32)
    W1p, W2p, b1rep, b2rep = prep_weights(inputs, cfg)
    x_pad = np.zeros((cfg.n_pad, cfg.f_in), np.float32)
    # node table in assignment-row order (matches relabeled gather indices
    # and the layer-2 table layout produced by the AllGather)
    x_pad[: cfg.n_nodes] = x[plan.ids.reshape(-1)]
    xT = np.ascontiguousarray(x_pad.T).astype(BF16)
    ident = np.eye(P, dtype=BF16)
    in_maps = []
    for c in range(cfg.n_cores):
        xo = np.zeros((cfg.shard_pad, cfg.f_in), np.float32)
        xo[: cfg.shard] = x[plan.ids[c]]
        d = plan.data[c]
        in_maps.append(
            {
                "xT": xT,
                "xTown": np.ascontiguousarray(xo.T).astype(BF16),
                "W1p": W1p,
                "W2p": W2p,
                "b1rep": b1rep,
                "b2rep": b2rep,
                "identity": ident,
                "IDXA": d["IDXA"],
                "S01": d["S01"],
                "S01T": d["S01T"],
            }
        )
    return in_maps


def kernel(**inputs) -> np.ndarray:
    cfg = Cfg()
    edge_index = np.asarray(inputs["edge_index"])
    plan = preprocess(edge_index, cfg)
    in_maps = make_in_maps(inputs, cfg, plan)
    nc = build_kernel(cfg, plan)

    from concourse.bass_utils import run_bass_kernel_spmd

    res = run_bass_kernel_spmd(nc, in_maps, core_ids=list(range(cfg.n_cores)))
    out = assemble_out([r["out"] for r in res.results], plan, cfg)
    return out


def assemble_out(outs: list, plan: Plan, cfg: Cfg) -> np.ndarray:
    """Un-permute the per-core output shards (rows are in balanced-assignment
    (tile, pos) order) back to node order."""
    full = np.empty((cfg.n_nodes, np.asarray(outs[0]).shape[1]), np.float32)
    for c, o in enumerate(outs):
        full[plan.ids[c]] = np.asarray(o, np.float32)
    return full



# revision 51
# speedup vs baseline: 1.1859x; 1.1859x over previous
"""GAT (2-layer, PyG-style) Trainium2 Bass kernel, 8-core SPMD.

Sharding: destinations are assigned to 8 cores x 49 tiles by a degree-balanced,
pool-preserving relabeling (see _balanced_assign): per-bucket A/B edge sums are
near-uniform (no max-over-cores chunk padding; CA=11, CB=6), and the relabeled
row is also the gather-table row for both layers. Each core:
  - computes the full node table h = x @ [W1 | W1@Asrc | W1@Adst] (replicated)
    in assignment order; table split at the int16 index boundary (pool-A nodes
    keep rows < 32768 so one A/B chunk split serves both layers),
  - keeps per-dst-tile attention scores (ad) resident in SBUF,
  - self-loops are NOT gathered: each tile's 128 self edges are consecutive
    table rows (T1own / T2sh), loaded by one contiguous HWDGE DMA per group
    and scattered with an fp8 identity block (no SWDGE descriptors),
  - gathers per-edge src rows with SWDGE dma_gather (8-chunk windows, one
    SWDGE queue; the ucode descriptor ring is hard-capped at 1024 descs,
    >1 queue hard-crashes the device),
  - computes per-edge dst scores via TensorE matmuls against host-built
    TRANSPOSED 0/1 scatter blocks (S01T) -- no dst gather,
  - computes edge scores  e = leakyrelu(as[src]+ad[dst]),  w = exp(e)
    (max-subtraction dropped: it cancels exactly in the softmax ratio),
  - aggregates  out[d] = (sum_e S01[e,d] * w_e * h[src_e]) / (sum_e S01[e,d] * w_e)
    via PSUM-accumulated TensorE matmuls against host-built 0/1 scatter blocks
    (lhsT dtype must stay uniform within one accumulation group -> fp8 identity),
  - applies bias+ELU, computes layer-2 node rows, AllGathers them (1.6MB/rank),
  - layer-2 dst scores ad2[dst_e] are computed during the layer-1 pass while
    each group's S01T blocks are still SBUF-resident (stashed in pss2_all), so
    the layer-2 pass skips all S01T loads (-15.4MB HBM traffic per core),
  - repeats the gather/aggregate phase for layer 2 and writes its output shard.

All indices / scatter blocks / paddings are host-precomputed per core and fed
as per-core input tensors, so one SPMD NEFF serves all 8 cores; the host
un-permutes the per-core output shards back to node order (assemble_out).
"""

import math
from dataclasses import dataclass, field

import numpy as np
import ml_dtypes

BF16 = ml_dtypes.bfloat16
FP8 = ml_dtypes.float8_e4m3

P = 128  # partitions / tile edge

# HW bisection: "A"=node tables only, "C"=+layer1 agg+allgather, "D"=full
BUILD_STAGE = "D"


@dataclass
class Cfg:
    n_nodes: int = 50000
    n_edges: int = 800000  # before self loops
    f_in: int = 128
    heads: int = 8
    hid: int = 32
    n_cores: int = 8
    group_tiles: int = 4  # dst tiles per gather group
    split: int = 32768  # int16 index split point
    neg_slope: float = 0.2
    window: int = 8  # gather chunks per dma_gather call
    nqueues: int = 1  # SWDGE queues
    single_packet: bool = True  # pack each gather call into one SDMA packet
    scratch: int = 16384  # dynamic DMA scratch bytes (ring = scratch/16 descs)
    nb: int = 4  # node-table tiles per batched DMA

    @property
    def shard(self):
        return self.n_nodes // self.n_cores

    @property
    def hc(self):
        return self.heads * self.hid  # 256

    @property
    def n_tiles(self):
        return math.ceil(self.shard / P)  # dst tiles per core

    @property
    def nt1(self):
        return math.ceil(self.n_nodes / P)  # node-table tiles

    @property
    def n_pad(self):
        return self.nt1 * P

    @property
    def shard_pad(self):
        return self.n_tiles * P


# ------------------------------------------------------------ host preprocess


@dataclass
class Plan:
    """Structure shared by all cores (uniform) + per-core tensor data."""

    CA: list = field(default_factory=list)  # A-chunks per tile (maxed over cores)
    CB: list = field(default_factory=list)
    groups: list = field(default_factory=list)  # per group: list of tile ids
    g_nA: list = field(default_factory=list)
    g_nB: list = field(default_factory=list)
    g_chunk0: list = field(default_factory=list)
    g_slot0: list = field(default_factory=list)  # bufh slot offsets (incl self)
    k_tot: int = 0  # A/B chunks (S01/IDXA entries)
    slot_tot: int = 0  # bufh slots incl per-tile self chunks
    data: list = field(default_factory=list)  # per-core input arrays
    ids: np.ndarray | None = None  # [n_cores, shard] node id per output row


def _wrap16(idx: np.ndarray) -> np.ndarray:
    """[n] -> [128, n/16] int16 gather-index layout (16-wrapped, x8 replicated)."""
    n = idx.shape[0]
    assert n % 16 == 0
    a = idx.astype(np.int16).reshape(n // 16, 16).T  # [16, n/16]
    return np.tile(a, (8, 1)).copy()


def _balanced_assign(dA: np.ndarray, dB: np.ndarray, cfg: Cfg):
    """Relabel nodes into balanced dst buckets (8 cores x 49 tiles).

    Nodes are snake-dealt (sorted desc by (dB, dA) in-degree) so per-bucket
    A/B edge sums are near-uniform, removing both the max-over-cores chunk
    padding and most per-tile rounding waste (k_tot 940 -> 880).

    The relabeled row is ALSO the gather-table row for both layers (the
    layer-2 table comes out of the AllGather in assignment order). To keep
    the int16 A/B table split valid for both layers, nodes with id < split
    must stay in rows < split: pool-A nodes are dealt into the low-row slots
    of each bucket, pool-B into the rest.
    """
    N = cfg.n_nodes
    nt, ncr = cfg.n_tiles, cfg.n_cores
    tail = cfg.shard - (nt - 1) * P
    nbuck = ncr * nt
    cap = np.full(nbuck, P, np.int64)
    cap[nt - 1 :: nt] = tail  # last tile of each core holds the remainder
    # global row of each bucket's position 0, and its pool-A slot quota
    r0 = np.array(
        [(b // nt) * cfg.shard + (b % nt) * P for b in range(nbuck)], np.int64
    )
    capA = np.clip(cfg.split - r0, 0, cap)
    capB = cap - capA

    node_core = np.empty(N, np.int32)
    node_tile = np.empty(N, np.int32)
    node_pos = np.empty(N, np.int32)
    order = np.lexsort((dA, dB))[::-1]

    def deal(nodes, quota, pos0):
        fill = np.zeros(nbuck, np.int64)
        active = [b for b in range(nbuck) if quota[b] > 0]
        i = 0
        fwd = True
        while i < len(nodes):
            seq = active if fwd else active[::-1]
            for b in seq:
                if i >= len(nodes):
                    break
                n = nodes[i]
                node_core[n] = b // nt
                node_tile[n] = b % nt
                node_pos[n] = pos0[b] + fill[b]
                fill[b] += 1
                i += 1
            active = [b for b in active if fill[b] < quota[b]]
            fwd = not fwd
        assert (fill == quota).all()

    deal(order[order < cfg.split], capA, np.zeros(nbuck, np.int64))
    deal(order[order >= cfg.split], capB, capA)
    return node_core, node_tile, node_pos


def preprocess(edge_index: np.ndarray, cfg: Cfg) -> Plan:
    N = cfg.n_nodes
    # self-loops are NOT part of the gathered edge streams: each tile's 128
    # self edges are consecutive table rows, loaded by one contiguous DMA
    # into dedicated "self" chunk slots (no per-row SWDGE descriptors)
    src = edge_index[0].astype(np.int64)
    dst = edge_index[1].astype(np.int64)

    plan = Plan()
    ncores = cfg.n_cores
    shard = cfg.shard
    cdiv = lambda a, b: -(-a // b)

    dA = np.bincount(dst[src < cfg.split], minlength=N)
    dB = np.bincount(dst[src >= cfg.split], minlength=N)
    node_core, node_tile, node_pos = _balanced_assign(dA, dB, cfg)
    # per-core output row order: node at (c, t, p) -> OUT row t*P + p
    ids = np.empty((ncores, shard), np.int64)
    rows = node_tile.astype(np.int64) * P + node_pos
    for c in range(ncores):
        m = node_core == c
        ids[c, rows[m]] = np.nonzero(m)[0]
    plan.ids = ids
    # gather tables (both layers) are stored in assignment-row order, so
    # gather indices are the relabeled rows; pool-preserving assignment keeps
    # src<split <=> row<split, so one A/B chunk split serves both layers
    row2 = node_core.astype(np.int64) * shard + rows
    src = row2[src]

    e_core = node_core[dst]
    e_tile = node_tile[dst]
    e_pos = node_pos[dst]
    per_core = []
    for c in range(ncores):
        m = e_core == c
        s_c = src[m]
        t_c = e_tile[m]
        p_c = e_pos[m]
        order = np.argsort(t_c * P + p_c, kind="stable")
        s_c, t_c, p_c = s_c[order], t_c[order], p_c[order]
        tiles = []
        for t in range(cfg.n_tiles):
            tm = t_c == t
            s_t, d_t = s_c[tm], p_c[tm]
            a = s_t < cfg.split
            tiles.append((s_t[a], d_t[a], s_t[~a], d_t[~a]))
        per_core.append(tiles)

    for t in range(cfg.n_tiles):
        plan.CA.append(max(cdiv(len(per_core[c][t][0]), P) for c in range(ncores)))
        plan.CB.append(max(cdiv(len(per_core[c][t][2]), P) for c in range(ncores)))

    for g0 in range(0, cfg.n_tiles, cfg.group_tiles):
        plan.groups.append(list(range(g0, min(g0 + cfg.group_tiles, cfg.n_tiles))))
    k = 0
    s = 0
    for g in plan.groups:
        plan.g_chunk0.append(k)
        plan.g_slot0.append(s)
        plan.g_nA.append(sum(plan.CA[t] for t in g))
        plan.g_nB.append(sum(plan.CB[t] for t in g))
        k += plan.g_nA[-1] + plan.g_nB[-1]
        s += plan.g_nA[-1] + plan.g_nB[-1] + len(g)  # + one self chunk per tile
    plan.k_tot = k
    plan.slot_tot = s

    for c in range(ncores):
        idxA = []
        s01 = np.zeros((plan.k_tot, P, P), dtype=FP8)
        for gi, g in enumerate(plan.groups):
            k0 = plan.g_chunk0[gi]
            nA = plan.g_nA[gi]
            a_off = 0
            b_off = 0
            gA_src, gB_src = [], []
            for t in g:
                sA, dA, sB, dB = per_core[c][t]
                la, lb = plan.CA[t] * P, plan.CB[t] * P
                sA_p = np.concatenate([sA, np.zeros(la - len(sA), np.int64)])
                sB_p = np.concatenate(
                    [sB - cfg.split, np.zeros(lb - len(sB), np.int64)]
                )
                gA_src.append(sA_p)
                gB_src.append(sB_p)
                if len(sA):
                    jj = np.arange(len(sA))
                    s01[k0 + a_off + jj // P, jj % P, dA] = 1.0
                if len(sB):
                    jj = np.arange(len(sB))
                    s01[k0 + nA + b_off + jj // P, jj % P, dB] = 1.0
                a_off += plan.CA[t]
                b_off += plan.CB[t]
            idxA.append(np.concatenate(gA_src + gB_src))
        # S01T (edge->dst score scatter) is built WITHOUT the fake entries:
        # it must map each edge slot to exactly its own dst column, otherwise
        # real edges sharing a slot with a fake entry get corrupted scores.
        s01T = np.ascontiguousarray(s01.transpose(0, 2, 1))  # [K, d, e]
        # fake entries so padded dst columns get denom > 0 (aggregation only)
        for gi, g in enumerate(plan.groups):
            k0 = plan.g_chunk0[gi]
            nA = plan.g_nA[gi]
            a_off = 0
            b_off = 0
            for t in g:
                width = min(cfg.shard - t * P, P)
                if width < P and plan.CA[t] + plan.CB[t] > 0:
                    kf = (k0 + a_off) if plan.CA[t] > 0 else (k0 + nA + b_off)
                    for d_pad in range(width, P):
                        s01[kf, (d_pad - width) % P, d_pad] = 1.0
                a_off += plan.CA[t]
                b_off += plan.CB[t]
        cat = lambda xs: (
            np.concatenate([_wrap16(x) for x in xs if len(x)], axis=1)
            if any(len(x) for x in xs)
            else np.zeros((128, 0), np.int16)
        )
        plan.data.append(
            {
                "IDXA": cat(idxA),
                "S01": np.ascontiguousarray(s01.transpose(1, 0, 2)),  # [128e,K,128d]
                "S01T": np.ascontiguousarray(s01T.transpose(1, 0, 2)),  # [128d,K,128e]
            }
        )
    return plan


def prep_weights(inputs: dict, cfg: Cfg):
    W1 = np.asarray(inputs["W1"], np.float32)
    a_s1 = np.asarray(inputs["att_src1"], np.float32)
    a_d1 = np.asarray(inputs["att_dst1"], np.float32)
    W2 = np.asarray(inputs["W2"], np.float32)
    a_s2 = np.asarray(inputs["att_src2"], np.float32)
    a_d2 = np.asarray(inputs["att_dst2"], np.float32)
    H, C = cfg.heads, cfg.hid
    W1r = W1.reshape(cfg.f_in, H, C)
    w1as = np.einsum("fhc,hc->fh", W1r, a_s1)
    w1ad = np.einsum("fhc,hc->fh", W1r, a_d1)
    W1p = np.concatenate([W1, w1as, w1ad], axis=1).astype(BF16)  # [F, HC+16]
    w2as = W2 @ a_s2[0]
    w2ad = W2 @ a_d2[0]
    W2p = np.concatenate([W2, w2as[:, None], w2ad[:, None]], axis=1).astype(BF16)
    b1rep = np.tile(np.asarray(inputs["b1"], np.float32)[None, :], (P, 1))
    b2rep = np.tile(np.asarray(inputs["b2"], np.float32)[None, :], (P, 1))
    return W1p, W2p, b1rep.astype(np.float32), b2rep.astype(np.float32)


# ---------------------------------------------------------------- bass kernel


def build_kernel(
    cfg: Cfg,
    plan: Plan,
    sim_1core: bool = False,
    phase_marks: dict | None = None,
    repeat: int = 1,  # unroll the whole body N times (bench-only: measures
    # device-serial time per body without per-dispatch RPC overhead)
):
    from contextlib import ExitStack

    import concourse.bacc as bacc
    import concourse.bass as bass
    import concourse.mybir as mybir
    import concourse.tile as tile

    fp32 = mybir.dt.float32
    bf16 = mybir.dt.bfloat16
    fp8 = mybir.dt.float8e4
    i16 = mybir.dt.int16
    AF = mybir.ActivationFunctionType
    OP = mybir.AluOpType

    HC = cfg.hc  # 256
    HCX = HC + 16
    H = cfg.heads
    HID = cfg.hid
    NTAB = cfg.n_pad
    SH = cfg.shard
    SHP = cfg.shard_pad
    T2W = 128  # layer-2 table row width (256B rows): [h2 | as2 | ad2 | pad]
    NKW2 = HC // P  # 2 chunks for the layer-2 prep matmul
    NT = cfg.n_tiles

    nc = bacc.Bacc(
        "TRN2",
        num_devices=1 if sim_1core else cfg.n_cores,
        num_swdge_queues=cfg.nqueues,
        dynamic_dma_scratch_size=cfg.scratch,
        name="gat8",
    )

    def mark(name):
        if phase_marks is not None:
            phase_marks[name] = len(nc.inst_map)

    xT = nc.dram_tensor("xT", [P, NTAB], bf16, kind="ExternalInput")
    xTown = nc.dram_tensor("xTown", [P, SHP], bf16, kind="ExternalInput")
    W1p = nc.dram_tensor("W1p", [cfg.f_in, HCX], bf16, kind="ExternalInput")
    W2p = nc.dram_tensor("W2p", [HC, HID + 2], bf16, kind="ExternalInput")
    b1rep = nc.dram_tensor("b1rep", [P, HC], fp32, kind="ExternalInput")
    b2rep = nc.dram_tensor("b2rep", [P, HID], fp32, kind="ExternalInput")
    identity = nc.dram_tensor("identity", [P, P], bf16, kind="ExternalInput")
    d0 = plan.data[0]
    IDXA = nc.dram_tensor("IDXA", list(d0["IDXA"].shape), i16, kind="ExternalInput")
    S01 = nc.dram_tensor("S01", [P, plan.k_tot, P], fp8, kind="ExternalInput")
    S01T = nc.dram_tensor("S01T", [P, plan.k_tot, P], fp8, kind="ExternalInput")
    OUT = nc.dram_tensor("out", [SH, HID], fp32, kind="ExternalOutput")

    with tile.TileContext(nc) as tc, ExitStack() as ctx:
        _regs = {}

        def nreg(v):
            if v not in _regs:
                _regs[v] = nc.gpsimd.to_reg(v)
            return _regs[v]

        sb = ctx.enter_context(tc.tile_pool(name="sb", bufs=2))
        sb1 = ctx.enter_context(tc.tile_pool(name="sb1", bufs=1))
        psA = ctx.enter_context(tc.tile_pool(name="psA", bufs=2, space="PSUM"))
        psB = ctx.enter_context(tc.tile_pool(name="psB", bufs=2, space="PSUM"))
        dram = ctx.enter_context(tc.tile_pool(name="dram", bufs=1, space="DRAM"))

        # node table split at the int16 index boundary so A-gathers can start
        # before the B half is written
        T1xA = dram.tile([cfg.split, 384], bf16, tag="T1xA")  # [h|as|ad|pad]
        T1xB = dram.tile([NTAB - cfg.split, 384], bf16, tag="T1xB")
        T1own = dram.tile([SHP, 384], bf16, tag="T1own")  # own-shard rows for
        # the contiguous self-chunk loads (static per-core address)
        T2sh = dram.tile([SH, T2W], bf16, tag="T2sh")
        T2full = dram.tile([cfg.n_nodes, T2W], bf16, tag="T2full")

        # constants
        w1_sb = sb1.tile([cfg.f_in, HCX], bf16, tag="w1")
        nc.sync.dma_start(w1_sb[:], W1p[:])
        w2_sb = sb1.tile([P, NKW2 * (HID + 2)], bf16, tag="w2")
        nc.sync.dma_start(
            w2_sb[:].rearrange("p (a n) -> p a n", a=NKW2),
            W2p[:].rearrange("(a p) n -> p a n", p=P),
        )
        w2_3 = w2_sb[:].rearrange("p (a n) -> p a n", a=NKW2)
        b1_sb = sb1.tile([P, HC], fp32, tag="b1")
        nc.sync.dma_start(b1_sb[:], b1rep[:])
        b2_sb = sb1.tile([P, HID], fp32, tag="b2")
        nc.sync.dma_start(b2_sb[:], b2rep[:])
        id_sb = sb1.tile([P, P], bf16, tag="id")
        nc.sync.dma_start(id_sb[:], identity[:])
        # fp8 identity for the self-chunk agg matmuls: they accumulate in the
        # same PSUM group as the fp8 scatter-block matmuls, and the lhsT dtype
        # must stay uniform within one accumulation group
        id8 = sb1.tile([P, P], fp8, tag="id8")
        nc.scalar.copy(id8[:], id_sb[:])

        # per-dst-tile attention scores, SBUF-resident
        adt1 = sb1.tile([P, NT * H], bf16, tag="adt1")  # ad per own node, layer 1
        adt2 = sb1.tile([P, NT], bf16, tag="adt2")  # ad2 per own node, layer 2
        # per-edge-chunk layer-2 dst scores ad2[dst_e], computed during the
        # layer-1 pass while the group's S01T blocks are still SBUF-resident
        # (saves reloading 15.4MB of S01T in the layer-2 pass)
        pss2_all = sb1.tile([P, plan.slot_tot], bf16, tag="pss2")

        # ---------------- phase 1: node table (batched DMAs) ----------------
        def phase1():
            NB = cfg.nb
            for i0 in range(0, cfg.nt1, NB):
                nb = min(NB, cfg.nt1 - i0)
                xt = sb.tile([P, NB * P], bf16, tag="xt")
                nc.sync.dma_start(xt[:, : nb * P], xT[:, i0 * P : (i0 + nb) * P])
                stg = sb.tile([P, NB * HCX], bf16, tag="stg1")
                stg3 = stg[:].rearrange("p (a e) -> p a e", a=NB)
                for j in range(nb):
                    pt = psA.tile([P, HCX], fp32, tag="p1")
                    nc.tensor.matmul(
                        out=pt[:],
                        lhsT=xt[:, j * P : (j + 1) * P],
                        rhs=w1_sb[:],
                        start=True,
                        stop=True,
                    )
                    if j % 2 == 0:
                        nc.vector.tensor_copy(stg3[:, j, :], pt[:])
                    else:
                        nc.scalar.copy(stg3[:, j, :], pt[:])
                r0 = i0 * P
                if r0 < cfg.split:
                    dst_tab = T1xA[r0 : r0 + nb * P, :HCX]
                else:
                    dst_tab = T1xB[r0 - cfg.split : r0 - cfg.split + nb * P, :HCX]
                # table writes on the Act HWDGE queue: the xt loads keep the
                # SP queue busy, so splitting queues overlaps load and store
                nc.scalar.dma_start(
                    dst_tab.rearrange("(a p) e -> p a e", p=P),
                    stg3[:, :nb, :],
                )

            # phase 1b: own-shard node rows -> T1own (for the contiguous
            # self-chunk loads) + dst scores (ad) kept in SBUF
            for i0 in range(0, NT, NB):
                nb = min(NB, NT - i0)
                xo = sb.tile([P, NB * P], bf16, tag="xo")
                nc.sync.dma_start(xo[:, : nb * P], xTown[:, i0 * P : (i0 + nb) * P])
                stg = sb.tile([P, NB * HCX], bf16, tag="stg1")
                stg3 = stg[:].rearrange("p (a e) -> p a e", a=NB)
                for j in range(nb):
                    pt = psA.tile([P, HCX], fp32, tag="p1")
                    nc.tensor.matmul(
                        out=pt[:],
                        lhsT=xo[:, j * P : (j + 1) * P],
                        rhs=w1_sb[:],
                        start=True,
                        stop=True,
                    )
                    if j % 2 == 0:
                        nc.vector.tensor_copy(stg3[:, j, :], pt[:])
                    else:
                        nc.scalar.copy(stg3[:, j, :], pt[:])
                    nc.scalar.copy(
                        adt1[:, (i0 + j) * H : (i0 + j + 1) * H],
                        pt[:, HC + H : HCX],
                    )
                nc.scalar.dma_start(
                    T1own[i0 * P : (i0 + nb) * P, :HCX].rearrange(
                        "(a p) e -> p a e", p=P
                    ),
                    stg3[:, :nb, :],
                )

        # ------------- layer-1 per-tile epilogue: bias, ELU, layer-2 rows ----
        def epilogue1_group(g, o1slab, r2slab3):
            """Group-level bias+ELU, then per-tile layer-2 row prep."""
            nt_g = len(g)
            w_cols = nt_g * HC
            y = sb.tile([P, nt_g * HC], fp32, tag="ep_y")
            nc.vector.tensor_tensor(
                out=y[:, :w_cols].rearrange("p (a e) -> p a e", a=nt_g),
                in0=o1slab[:, :w_cols].rearrange("p (a e) -> p a e", a=nt_g),
                in1=b1_sb[:].rearrange("p (u e) -> p u e", u=1).to_broadcast(
                    [P, nt_g, HC]
                ),
                op=OP.add,
            )
            mn = sb.tile([P, nt_g * HC], fp32, tag="ep_mn")
            nc.vector.tensor_scalar_min(mn[:, :w_cols], y[:, :w_cols], 0.0)
            nc.scalar.activation(mn[:, :w_cols], mn[:, :w_cols], AF.Exp)
            nc.vector.tensor_scalar_max(y[:, :w_cols], y[:, :w_cols], 0.0)
            nc.vector.tensor_tensor(
                out=y[:, :w_cols], in0=y[:, :w_cols], in1=mn[:, :w_cols], op=OP.add
            )
            elu_bf = sb.tile([P, nt_g * HC], bf16, tag="ep_bf")
            nc.vector.tensor_scalar_add(elu_bf[:, :w_cols], y[:, :w_cols], -1.0)
            for jj, t in enumerate(g):
                eluT = sb.tile([P, HC], bf16, tag="ep_eT")
                for j in range(NKW2):
                    ptT = psB.tile([P, P], bf16, tag="p2")
                    nc.tensor.transpose(
                        out=ptT[:],
                        in_=elu_bf[:, jj * HC + j * P : jj * HC + (j + 1) * P],
                        identity=id_sb[:],
                    )
                    nc.scalar.copy(eluT[:, j * P : (j + 1) * P], ptT[:])
                p2 = psB.tile([P, HID + 2], fp32, tag="p2")
                for j in range(NKW2):
                    nc.tensor.matmul(
                        out=p2[:],
                        lhsT=eluT[:, j * P : (j + 1) * P],
                        rhs=w2_3[:, j, :],
                        start=(j == 0),
                        stop=(j == NKW2 - 1),
                    )
                nc.vector.tensor_copy(r2slab3[:, jj, :], p2[:])
                nc.scalar.copy(adt2[:, t : t + 1], p2[:, HID + 1 : HID + 2])

        # ---------------- shared gather/aggregate phase ----------------
        qctr = [0]

        def agg_layer(layer):
            elem_h = 384 if layer == 1 else T2W
            nhead = H if layer == 1 else 1
            rhs_w = HC if layer == 1 else HID  # message width

            for gi, g in enumerate(plan.groups):
                nA, nBg = plan.g_nA[gi], plan.g_nB[gi]
                ng = nA + nBg  # A/B chunks (have S01 blocks + gather idxs)
                nt_g = len(g)
                ngs = ng + nt_g  # + one self chunk per tile (slots at the end)
                k0 = plan.g_chunk0[gi]
                s0g = plan.g_slot0[gi]

                bufh = sb.tile([P, ngs * elem_h], bf16, tag="bufh")
                bufh3 = bufh[:].rearrange("p (k e) -> p k e", e=elem_h)

                def win_gather(out3, table_ap, idx_tile, c0, n_chunks, elem):
                    for w0 in range(0, n_chunks, cfg.window):
                        wn = min(cfg.window, n_chunks - w0)
                        nc.gpsimd.dma_gather(
                            out_ap=out3[:, c0 + w0 : c0 + w0 + wn, :],
                            in_ap=table_ap,
                            idxs_ap=idx_tile[:, (c0 + w0) * 8 : (c0 + w0 + wn) * 8],
                            num_idxs=wn * P,
                            num_idxs_reg=nreg(wn * P),
                            elem_size=elem,
                            queue_num=qctr[0] % cfg.nqueues,
                            single_packet=cfg.single_packet,
                        )
                        qctr[0] += 1

                ih = sb.tile([P, ng * 8], i16, tag="ia")
                a0 = sum(plan.g_nA[j] + plan.g_nB[j] for j in range(gi)) * 8
                nc.sync.dma_start(ih[:], IDXA[:, a0 : a0 + ng * 8])
                tabA = T1xA if layer == 1 else T2full
                tabB = T1xB if layer == 1 else T2full[cfg.split :, :]
                if nA:
                    win_gather(bufh3, tabA[:, :] if layer == 1 else tabA[:, :], ih, 0, nA, elem_h)
                if nBg:
                    win_gather(bufh3, tabB[:, :] if layer == 1 else tabB, ih, nA, nBg, elem_h)
                # self chunks: contiguous own-shard rows, plain HWDGE DMA
                t0g = g[0]
                if layer == 1:
                    nc.scalar.dma_start(
                        bufh3[:, ng : ng + nt_g, :],
                        T1own[t0g * P : (t0g + nt_g) * P, :].rearrange(
                            "(a p) e -> p a e", p=P
                        ),
                    )
                else:
                    for jj, t in enumerate(g):
                        width = min(SH - t * P, P)
                        nc.scalar.dma_start(
                            bufh3[:width, ng + jj, :],
                            T2sh[t * P : t * P + width, :],
                        )

                s01_sb = sb.tile([P, ng * P], fp8, tag="s01")
                nc.sync.dma_start(s01_sb[:], S01[:, k0 : k0 + ng, :])
                s01_3 = s01_sb[:].rearrange("p (k d) -> p k d", d=P)
                if layer == 1:
                    s01T_sb = sb.tile([P, ng * P], fp8, tag="s01T")
                    nc.scalar.dma_start(s01T_sb[:], S01T[:, k0 : k0 + ng, :])
                    s01T_3 = s01T_sb[:].rearrange("p (k e) -> p k e", e=P)

                    # dst scores via S01T matmuls: psS[e, k*nh:] = S01T_k^T @ adt[t]
                    # (self chunks: identity scatter -> a PE copy of adt1[t])
                    pss = psA.tile([P, ngs * nhead], fp32, tag="pagg")
                    a_off, b_off = 0, 0
                    for jj, t in enumerate(g):
                        chunks = [a_off + j for j in range(plan.CA[t])] + [
                            nA + b_off + j for j in range(plan.CB[t])
                        ]
                        for k in chunks:
                            nc.tensor.matmul(
                                out=pss[:, k * nhead : (k + 1) * nhead],
                                lhsT=s01T_3[:, k, :],
                                rhs=adt1[:, t * H : (t + 1) * H],
                                start=True,
                                stop=True,
                            )
                        kS = ng + jj
                        nc.tensor.matmul(
                            out=pss[:, kS * nhead : (kS + 1) * nhead],
                            lhsT=id_sb[:],
                            rhs=adt1[:, t * H : (t + 1) * H],
                            start=True,
                            stop=True,
                        )
                        a_off += plan.CA[t]
                        b_off += plan.CB[t]
                    pss_ap = pss[:].rearrange("p (k h) -> p k h", h=nhead)
                else:
                    # layer-2 dst scores were stashed during the layer-1 pass
                    pss_ap = pss2_all[:, s0g : s0g + ngs].rearrange(
                        "p (k h) -> p k h", h=1
                    )

                # scores: s = as[src] + ad[dst]; w = exp(leakyrelu(s))
                nsc = ngs * nhead
                s_f = sb.tile([P, nsc], fp32, tag="s_f")
                if layer == 1:
                    as_ap = bufh3[:, :, HC : HC + H]
                else:
                    as_ap = bufh3[:, :, HID : HID + 1]
                nc.vector.tensor_tensor(
                    out=s_f[:],
                    in0=as_ap,
                    in1=pss_ap,
                    op=OP.add,
                )
                s_lr = sb.tile([P, nsc], fp32, tag="s_lr")
                nc.scalar.mul(s_lr[:], s_f[:], cfg.neg_slope)
                nc.vector.tensor_tensor(out=s_f[:], in0=s_lr[:], in1=s_f[:], op=OP.max)
                # w written pair-duplicated so the premultiply's broadcast
                # operand has an innermost stride-1 pair (DVE 2x mode)
                w2x = sb.tile([P, nsc * 2], bf16, tag="w2x")
                nc.scalar.activation(
                    w2x[:].rearrange("p (k two) -> p k two", two=2),
                    s_f[:].rearrange("p (k u) -> p k u", u=1).to_broadcast(
                        [P, nsc, 2]
                    ),
                    AF.Exp,
                )
                # copy w into the padding columns of the gathered rows so the
                # aggregation matmul also produces softmax denominators
                wcol = 256 if layer == 1 else 33
                nc.scalar.copy(
                    bufh3[:, :, wcol : wcol + nhead],
                    w2x[:].rearrange("p (k two) -> p k two", two=2)[:, :, 0].rearrange(
                        "p (k h) -> p k h", h=nhead
                    ),
                )

                # premultiply gathered message rows by w (in place, 2x mode)
                mw_io = bufh3[:, :, : (HC if layer == 1 else HID)].rearrange(
                    "p k (h x two) -> p k h x two", h=nhead, two=2
                )
                w_b = w2x[:].rearrange(
                    "p (k h u two) -> p k h u two", h=nhead, u=1, two=2
                ).to_broadcast([P, ngs, nhead, HID // 2, 2])
                nc.vector.tensor_tensor(out=mw_io, in0=mw_io, in1=w_b, op=OP.mult)

                a_off, b_off = 0, 0
                rw = wcol + nhead  # matmul rhs width incl junk + w cols
                nt_g = len(g)
                if layer == 1:
                    r2slab = sb.tile([P, nt_g * (HID + 2)], fp32, tag="r2slab")
                    r2slab3 = r2slab[:].rearrange("p (a e) -> p a e", a=nt_g)
                    o1slab = sb.tile([P, nt_g * HC], fp32, tag="o1slab")
                    o1slab3 = o1slab[:].rearrange(
                        "p (a h c) -> p a h c", a=nt_g, c=HID
                    )
                else:
                    oslab = sb.tile([P, nt_g * HID], fp32, tag="oslab")
                    oslab3 = oslab[:].rearrange("p (a e) -> p a e", a=nt_g)
                for jj, t in enumerate(g):
                    pt = psA.tile([P, rw], fp32, tag="pagg")
                    chunks = [a_off + j for j in range(plan.CA[t])] + [
                        nA + b_off + j for j in range(plan.CB[t])
                    ]
                    for ci, k in enumerate(chunks):
                        nc.tensor.matmul(
                            out=pt[:],
                            lhsT=s01_3[:, k, :],
                            rhs=bufh3[:, k, :rw],
                            start=(ci == 0),
                            stop=False,
                        )
                    # self chunk: identity scatter (slot p -> dst col p); for
                    # the layer-2 tail tile only `width` rows of T2sh exist,
                    # so contract over [:width] to exclude stale SBUF rows
                    kS = ng + jj
                    width = min(SH - t * P, P)
                    wlim = width if (layer == 2 and width < P) else P
                    nc.tensor.matmul(
                        out=pt[:],
                        lhsT=id8[:wlim, :],
                        rhs=bufh3[:wlim, kS, :rw],
                        start=(len(chunks) == 0),
                        stop=True,
                    )
                    a_off += plan.CA[t]
                    b_off += plan.CB[t]

                    den_r = sb.tile([P, nhead], fp32, tag="denr")
                    nc.vector.reciprocal(den_r[:], pt[:, wcol : wcol + nhead])
                    if layer == 1:
                        nc.vector.tensor_tensor(
                            out=o1slab3[:, jj, :, :],
                            in0=pt[:, :rhs_w].rearrange("p (h c) -> p h c", c=HID),
                            in1=den_r[:].to_broadcast([P, nhead, HID]),
                            op=OP.mult,
                        )
                    else:
                        nc.vector.tensor_tensor(
                            out=oslab3[:, jj, :],
                            in0=pt[:, :HID],
                            in1=den_r[:].to_broadcast([P, HID]),
                            op=OP.mult,
                        )
                        nc.vector.tensor_tensor(
                            out=oslab3[:, jj, :],
                            in0=oslab3[:, jj, :],
                            in1=b2_sb[:],
                            op=OP.add,
                        )
                # batched per-group stores
                t0g = g[0]
                rows = min(SH - t0g * P, nt_g * P)
                if layer == 1:
                    epilogue1_group(g, o1slab, r2slab3)
                    # stash layer-2 dst scores for this group's chunks while
                    # its S01T blocks are still resident
                    p2s = psB.tile([P, ng], fp32, tag="p2")
                    a_off2, b_off2 = 0, 0
                    for jj, t in enumerate(g):
                        chunks2 = [a_off2 + j for j in range(plan.CA[t])] + [
                            nA + b_off2 + j for j in range(plan.CB[t])
                        ]
                        for k in chunks2:
                            nc.tensor.matmul(
                                out=p2s[:, k : k + 1],
                                lhsT=s01T_3[:, k, :],
                                rhs=adt2[:, t : t + 1],
                                start=True,
                                stop=True,
                            )
                        a_off2 += plan.CA[t]
                        b_off2 += plan.CB[t]
                        # self chunk: ad2[dst=self] is just adt2[t] itself
                        nc.scalar.copy(
                            pss2_all[:, s0g + ng + jj : s0g + ng + jj + 1],
                            adt2[:, t : t + 1],
                        )
                    nc.scalar.copy(pss2_all[:, s0g : s0g + ng], p2s[:])
                    r2bf = sb.tile([P, nt_g * (HID + 2)], bf16, tag="r2bf")
                    nc.vector.tensor_copy(r2bf[:], r2slab[:])
                    r2bf3 = r2bf[:].rearrange("p (a e) -> p a e", a=nt_g)
                    if rows % P == 0:
                        nc.sync.dma_start(
                            T2sh[t0g * P : t0g * P + rows, : HID + 2].rearrange(
                                "(a p) e -> p a e", p=P
                            ),
                            r2bf3[:, : rows // P, :],
                        )
                    else:
                        nfull = rows // P
                        if nfull:
                            nc.sync.dma_start(
                                T2sh[t0g * P : t0g * P + nfull * P, : HID + 2].rearrange(
                                    "(a p) e -> p a e", p=P
                                ),
                                r2bf3[:, :nfull, :],
                            )
                        rem = rows - nfull * P
                        nc.sync.dma_start(
                            T2sh[
                                t0g * P + nfull * P : t0g * P + rows, : HID + 2
                            ],
                            r2bf3[:rem, nfull, :],
                        )
                else:
                    if rows % P == 0:
                        nc.sync.dma_start(
                            OUT[t0g * P : t0g * P + rows, :].rearrange(
                                "(a p) e -> p a e", p=P
                            ),
                            oslab3[:, : rows // P, :],
                        )
                    else:
                        nfull = rows // P
                        if nfull:
                            nc.sync.dma_start(
                                OUT[t0g * P : t0g * P + nfull * P, :].rearrange(
                                    "(a p) e -> p a e", p=P
                                ),
                                oslab3[:, :nfull, :],
                            )
                        rem = rows - nfull * P
                        nc.sync.dma_start(
                            OUT[t0g * P + nfull * P : t0g * P + rows, :],
                            oslab3[:rem, nfull, :],
                        )

        for _rep in range(repeat):
            phase1()
            mark("node_tables")
            if BUILD_STAGE != "A":
                agg_layer(1)
            mark("agg1")

            if BUILD_STAGE in ("C", "D"):
                if cfg.n_cores > 1 and not sim_1core:
                    nc.gpsimd.collective_compute(
                        "AllGather",
                        OP.bypass,
                        replica_groups=[list(range(cfg.n_cores))],
                        ins=[T2sh.opt()],
                        outs=[T2full.opt()],
                    )
                else:
                    nc.sync.dma_start(T2full[:SH, :], T2sh[:, :])
            mark("allgather")

            if BUILD_STAGE == "D":
                agg_layer(2)
                mark("agg2")
            else:
                stg0 = sb.tile([P, HID], fp32, tag="dumm")
                for t in range(NT):
                    rows = min(SH - t * P, P)
                    nc.vector.tensor_copy(stg0[:rows, :], b2_sb[:rows, :])
                    nc.sync.dma_start(OUT[t * P : t * P + rows, :], stg0[:rows, :])

    nc.compile()
    return nc


# -------------------------------------------------------------------- driver


def make_in_maps(inputs: dict, cfg: Cfg, plan: Plan):
    x = np.asarray(inputs["x"], np.float32)
    W1p, W2p, b1rep, b2rep = prep_weights(inputs, cfg)
    x_pad = np.zeros((cfg.n_pad, cfg.f_in), np.float32)
    # node table in assignment-row order (matches relabeled gather indices
    # and the layer-2 table layout produced by the AllGather)
    x_pad[: cfg.n_nodes] = x[plan.ids.reshape(-1)]
    xT = np.ascontiguousarray(x_pad.T).astype(BF16)
    ident = np.eye(P, dtype=BF16)
    in_maps = []
    for c in range(cfg.n_cores):
        xo = np.zeros((cfg.shard_pad, cfg.f_in), np.float32)
        xo[: cfg.shard] = x[plan.ids[c]]
        d = plan.data[c]
        in_maps.append(
            {
                "xT": xT,
                "xTown": np.ascontiguousarray(xo.T).astype(BF16),
                "W1p": W1p,
                "W2p": W2p,
                "b1rep": b1rep,
                "b2rep": b2rep,
                "identity": ident,
                "IDXA": d["IDXA"],
                "S01": d["S01"],
                "S01T": d["S01T"],
            }
        )
    return in_maps


def kernel(**inputs) -> np.ndarray:
    cfg = Cfg()
    edge_index = np.asarray(inputs["edge_index"])
    plan = preprocess(edge_index, cfg)
    in_maps = make_in_maps(inputs, cfg, plan)
    nc = build_kernel(cfg, plan)

    from concourse.bass_utils import run_bass_kernel_spmd

    res = run_bass_kernel_spmd(nc, in_maps, core_ids=list(range(cfg.n_cores)))
    out = assemble_out([r["out"] for r in res.results], plan, cfg)
    return out


def assemble_out(outs: list, plan: Plan, cfg: Cfg) -> np.ndarray:
    """Un-permute the per-core output shards (rows are in balanced-assignment
    (tile, pos) order) back to node order."""
    full = np.empty((cfg.n_nodes, np.asarray(outs[0]).shape[1]), np.float32)
    for c, o in enumerate(outs):
        full[plan.ids[c]] = np.asarray(o, np.float32)
    return full

